# revision 11
# baseline (speedup 1.0000x reference)
"""Trainium2 Bass kernel for nn_Attention_73486890434886.

Gated 8-head attention (head_dim 32) with a full [8, 2048, 2048] attention
bias, batch 1, q_len = kv_len = 2048, fused QG / KV projections and a gated
output projection.

Strategy (8 NeuronCores, SPMD, no collectives):
  - Shard the 2048 q rows across the 8 cores (256 rows each).  Every core
    computes all 8 heads for its q-slice; kv-side projections are replicated
    (cheap) which removes the output all-reduce entirely.
  - All attention math is done in a "transposed" orientation so no on-device
    transposes are needed: logits^T [kv, q] = k^T-stationary x q^T-moving,
    the bias is pre-transposed on the host and injected into PSUM with an
    identity-stationary matmul, exp runs on the scalar engine, and attn@v
    consumes exp(logits^T) as the matmul moving operand producing
    attn_out^T [c, q].  Softmax denominators come from a ones-column matmul
    packed into the same PSUM accumulator bank.
  - bf16 everywhere on the TensorEngine (fp32 accumulation in PSUM), f32 for
    softmax/statistics on ACT/DVE.
"""

import numpy as np
import ml_dtypes

import concourse.bass as bass
import concourse.mybir as mybir
import concourse.tile as tile
from concourse import bacc
from concourse.bass_utils import run_bass_kernel_spmd

BF16 = ml_dtypes.bfloat16

# Problem shapes (hardcoded per the task statement).
B, QL, KVL, D, H, C, O = 1, 2048, 2048, 256, 8, 32, 256
NCORES = 8
QS = QL // NCORES          # 256 q rows per core
NKC = KVL // 128           # 16 kv chunks of 128
NG = 2                     # head groups (0-3, 4-7)
HPG = H // NG              # heads per group = 4

f32 = mybir.dt.float32
bf16 = mybir.dt.bfloat16


# ---------------------------------------------------------------------------
# Host-side packing: everything is laid out partition-major so every DMA is a
# straight contiguous copy.
# ---------------------------------------------------------------------------

def _pack_shared(inputs):
    """Pack the tensors that are identical on every core."""
    kv = np.asarray(inputs["kv_inputs"], np.float32)[0]        # [KVL, D]
    qg_w = np.asarray(inputs["qg_weights"], np.float32)[:, 0]  # [D, H, 2C]
    qg_b = np.asarray(inputs["qg_bias"], np.float32)[0, :, 0]  # [H, 2C]
    kv_w = np.asarray(inputs["kv_weights"], np.float32)[:, 0]  # [D, H, 2C]
    kv_b = np.asarray(inputs["kv_bias"], np.float32)[0, :, 0]  # [H, 2C]
    o_w = np.asarray(inputs["o_weights"], np.float32)[0]       # [H, C, O]
    o_b = np.asarray(inputs["o_bias"], np.float32)[:, 0]       # [O]

    scale = C ** -0.5

    def tiles2(w):
        # w [D, H, C] -> [128 p, NG tile, 2 kc, 128 m] with m = h'*C + c
        t = w.reshape(D, NG, HPG * C)          # [D, g, m]
        t = t.transpose(1, 0, 2)               # [g, D, m]
        t = t.reshape(NG, 2, 128, HPG * C)     # [g, kc, p, m]
        return t.transpose(2, 0, 1, 3)         # [p, g, kc, m]

    # Per-head zero-padded q weights: column block 32h'..32h'+32 of stationary
    # tile h carries w_q for head h, everything else is 0.  The resulting
    # projection output qT_pad[:, h, :] is zero outside head h's 32 c-rows, so
    # the logits matmul can contract over the full 128 partitions of the
    # packed k tile without mixing heads (row-group packing is broken on HW).
    wq_full = qg_w[:, :, :C] * scale           # [D, H, C]
    wq_pad = np.zeros((D, H, 128), np.float32)
    for h in range(H):
        hp = h % HPG
        wq_pad[:, h, 32 * hp:32 * hp + 32] = wq_full[:, h, :]
    wq_pad = wq_pad.reshape(2, 128, H, 128).transpose(1, 2, 0, 3)  # [128, H, kc, 128]

    wg = tiles2(qg_w[:, :, C:])                # [128, 2, 2, 128]
    wk = tiles2(kv_w[:, :, :C])                # [128, 2, 2, 128]
    wv = kv_w[:, :, C:].reshape(D, H * C)      # [D, 256]
    wv = wv.reshape(2, 128, H * C)             # [kc, p, m]
    wv = wv.transpose(1, 0, 2)                 # [128, 2, 256]

    # Per-head padded q bias: column h has the (scaled) q bias in rows
    # 32h'..32h'+32 and zeros elsewhere so the zero rows of qT_pad stay zero.
    qb_full = qg_b[:, :C] * scale              # [H, C]
    qbp = np.zeros((128, H), np.float32)
    for h in range(H):
        hp = h % HPG
        qbp[32 * hp:32 * hp + 32, h] = qb_full[h]
    gbn = (-qg_b[:, C:]).reshape(NG, 128).T                # [128, 2]
    kb = kv_b[:, :C].reshape(NG, 128).T                    # [128, 2]
    vbb = np.broadcast_to(kv_b[:, C:].reshape(1, H * C), (128, H * C)).copy()

    ow = o_w.reshape(H * C, O)                 # [(h,c), o]
    ow = ow.reshape(NG, 128, 2, 128)           # [g, p, t, o]
    ow = ow.transpose(1, 0, 2, 3)              # [128, g, t, o]
    ob = o_b.reshape(2, 128).T                 # [128, 2]

    kviT = kv.T.reshape(2, 128, KVL).transpose(1, 0, 2)    # [128, 2, KVL]

    iden = np.eye(128, dtype=np.float32)
    ind = np.zeros((128, 128), np.float32)
    for m in range(128):
        ind[32 * (m // 32), m] = 1.0

    return {
        "kviT": kviT.astype(BF16),
        "wqp": np.ascontiguousarray(wq_pad).astype(BF16),
        "wg": np.ascontiguousarray(wg).astype(BF16),
        "wk": np.ascontiguousarray(wk).astype(BF16),
        "wv": np.ascontiguousarray(wv).astype(BF16),
        "ow": np.ascontiguousarray(ow).astype(BF16),
        "qbp": np.ascontiguousarray(qbp).astype(np.float32),
        "gbn": np.ascontiguousarray(gbn).astype(np.float32),
        "kb": np.ascontiguousarray(kb).astype(np.float32),
        "vbb": vbb.astype(np.float32),
        "ob": np.ascontiguousarray(ob).astype(np.float32),
        "iden": iden.astype(BF16),
        "ind": ind.astype(BF16),
    }


def _pack_core(inputs, core):
    """Pack the q-shard-specific tensors for one core."""
    qs = core * QS
    q = np.asarray(inputs["q_inputs"], np.float32)[0]          # [QL, D]
    bias = np.asarray(inputs["bias"], np.float32)[0]           # [H, QL, KVL]

    qiT = q[qs:qs + QS].T.reshape(2, 128, QS).transpose(1, 0, 2)  # [128, 2, QS]

    b = bias[:, qs:qs + QS, :]                   # [H, QS, KVL]
    b = b.reshape(NG, HPG, QS, NKC, 128)         # [g, h', q, c, p]
    b = b.transpose(4, 0, 3, 1, 2)               # [p, g, c, h', q]
    bT = b.reshape(128, NG, NKC, HPG * QS)       # [128, 2, 16, 1024]

    return {
        "qiT": np.ascontiguousarray(qiT).astype(BF16),
        "bT": np.ascontiguousarray(bT).astype(BF16),
    }


def make_in_maps(inputs):
    shared = _pack_shared(inputs)
    maps = []
    for core in range(NCORES):
        m = dict(shared)
        m.update(_pack_core(inputs, core))
        maps.append(m)
    return maps


def gather_output(results):
    """results[i]['out'] is [2, 128, QS] (o-tile, o, q) -> full [1, QL, O]."""
    out = np.empty((1, QL, O), np.float32)
    for core, res in enumerate(results):
        oT = np.asarray(res["out"], np.float32).reshape(O, QS)  # [o, q]
        out[0, core * QS:(core + 1) * QS, :] = oT.T
    return out


# ---------------------------------------------------------------------------
# Numpy mimic of the device dataflow (for validating the packing / orientation
# algebra without hardware).  Mirrors each device matmul 1:1.
# ---------------------------------------------------------------------------

def numpy_model(inputs):
    maps = make_in_maps(inputs)
    results = []
    for core in range(NCORES):
        m = {k: np.asarray(v, np.float32) for k, v in maps[core].items()}
        kviT, qiT, bT = m["kviT"], m["qiT"], m["bT"]
        wqp, wg, wk, wv, ow = m["wqp"], m["wg"], m["wk"], m["wv"], m["ow"]
        qbp, gbn, kb, vbb, ob = m["qbp"], m["gbn"], m["kb"], m["vbb"], m["ob"]
        ind = m["ind"]

        # qg projection -> per-head zero-padded qT_pad and sigT
        qTp = np.zeros((128, H, QS), np.float32)
        for h in range(H):
            acc = np.zeros((128, QS), np.float32)
            for kc in range(2):
                acc += wqp[:, h, kc, :].T @ qiT[:, kc, :]
            qTp[:, h, :] = _bf(acc + qbp[:, h:h + 1])
        sigT = np.zeros((128, 2, QS), np.float32)
        for g in range(NG):
            acc = np.zeros((128, QS), np.float32)
            for kc in range(2):
                acc += wg[:, g, kc, :].T @ qiT[:, kc, :]
            e = np.exp(-acc + gbn[:, g:g + 1])
            sigT[:, g, :] = 1.0 / (1.0 + e)

        # kT projection [128, 2, KVL]
        kT = np.zeros((128, 2, KVL), np.float32)
        for t in range(2):
            acc = np.zeros((128, KVL), np.float32)
            for kc in range(2):
                acc += wk[:, t, kc, :].T @ kviT[:, kc, :]
            kT[:, t, :] = _bf(acc + kb[:, t:t + 1])

        # v projection [128, 16, 256]
        v = np.zeros((128, NKC, H * C), np.float32)
        for c in range(NKC):
            acc = np.zeros((128, H * C), np.float32)
            for kc in range(2):
                acc += kviT[:, kc, c * 128:(c + 1) * 128].T @ wv[:, kc, :]
            v[:, c, :] = _bf(acc + vbb)

        attn_gT = np.zeros((128, 2, QS), np.float32)
        for g in range(NG):
            accb = np.zeros((128, 512), np.float32)   # numer | rowsum bank
            for c in range(NKC):
                lt = np.zeros((128, HPG, QS), np.float32)
                for b2 in range(2):
                    lt[:, 2 * b2:2 * b2 + 2, :] += bT[:, g, c, 512 * b2:512 * (b2 + 1)].reshape(128, 2, QS)
                for hp in range(HPG):
                    h = HPG * g + hp
                    lt[:, hp, :] += kT[:, g, c * 128:(c + 1) * 128].T @ qTp[:, h, :]
                et = _bf(np.exp(lt))
                for hp in range(HPG):
                    h = HPG * g + hp
                    accb[32 * hp:32 * hp + 32, 0:QS] += v[:, c, 32 * h:32 * h + 32].T @ et[:, hp, :]
                    accb[32 * hp:32 * hp + 1, QS:2 * QS] += np.ones((128, 1), np.float32).T @ et[:, hp, :]
            rsg = np.zeros((128, QS), np.float32)
            for hp in range(HPG):
                rsg[32 * hp] = _bf(accb[32 * hp, QS:2 * QS])
            rsb = ind.T @ rsg
            recipB = 1.0 / rsb
            attn_gT[:, g, :] = _bf(accb[:, 0:QS] * sigT[:, g, :] * recipB)

        outT = np.zeros((2, 128, QS), np.float32)
        for t in range(2):
            acc = np.zeros((128, QS), np.float32)
            for g in range(2):
                acc += ow[:, g, t, :].T @ attn_gT[:, g, :]
            outT[t] = acc + ob[:, t:t + 1]
        results.append({"out": outT})
    return gather_output(results)


def _bf(x):
    return x.astype(BF16).astype(np.float32)


# ---------------------------------------------------------------------------
# Device kernel builder
# ---------------------------------------------------------------------------

def build_kernel():
    nc = bacc.Bacc("TRN2", target_bir_lowering=False, debug=False)

    p_qiT = nc.declare_dram_parameter("qiT", [128, 2, QS], bf16, False)
    p_kviT = nc.declare_dram_parameter("kviT", [128, 2, KVL], bf16, False)
    p_bT = nc.declare_dram_parameter("bT", [128, NG, NKC, HPG * QS], bf16, False)
    p_wqp = nc.declare_dram_parameter("wqp", [128, H, 2, 128], bf16, False)
    p_wg = nc.declare_dram_parameter("wg", [128, 2, 2, 128], bf16, False)
    p_wk = nc.declare_dram_parameter("wk", [128, 2, 2, 128], bf16, False)
    p_wv = nc.declare_dram_parameter("wv", [128, 2, 256], bf16, False)
    p_ow = nc.declare_dram_parameter("ow", [128, 2, 2, 128], bf16, False)
    p_qbp = nc.declare_dram_parameter("qbp", [128, H], f32, False)
    p_gbn = nc.declare_dram_parameter("gbn", [128, 2], f32, False)
    p_kb = nc.declare_dram_parameter("kb", [128, 2], f32, False)
    p_vbb = nc.declare_dram_parameter("vbb", [128, 256], f32, False)
    p_ob = nc.declare_dram_parameter("ob", [128, 2], f32, False)
    p_iden = nc.declare_dram_parameter("iden", [128, 128], bf16, False)
    p_ind = nc.declare_dram_parameter("ind", [128, 128], bf16, False)
    p_out = nc.declare_dram_parameter("out", [2, 128, QS], f32, True)

    Exp = mybir.ActivationFunctionType.Exp
    ADD = mybir.AluOpType.add
    MUL = mybir.AluOpType.mult

    with tile.TileContext(nc) as tc:
        with (
            tc.tile_pool(name="sb", bufs=1) as sb,
            tc.tile_pool(name="etp", bufs=2) as etp,
            tc.tile_pool(name="tmp", bufs=2) as tmp,
            tc.tile_pool(name="ps", bufs=2, space="PSUM") as ps,
            tc.tile_pool(name="pswork", bufs=2, space="PSUM") as pswork,
        ):
            # ---- resident SBUF loads (all contiguous DMAs) ----
            s_qiT = sb.tile([128, 2, QS], bf16)
            nc.sync.dma_start(out=s_qiT, in_=p_qiT[:])
            s_kviT = sb.tile([128, 2, KVL], bf16)
            nc.sync.dma_start(out=s_kviT, in_=p_kviT[:])
            s_wqp = sb.tile([128, H, 2, 128], bf16)
            nc.sync.dma_start(out=s_wqp, in_=p_wqp[:])
            s_wg = sb.tile([128, 2, 2, 128], bf16)
            nc.sync.dma_start(out=s_wg, in_=p_wg[:])
            s_wk = sb.tile([128, 2, 2, 128], bf16)
            nc.sync.dma_start(out=s_wk, in_=p_wk[:])
            s_wv = sb.tile([128, 2, 256], bf16)
            nc.sync.dma_start(out=s_wv, in_=p_wv[:])
            s_ow = sb.tile([128, 2, 2, 128], bf16)
            nc.sync.dma_start(out=s_ow, in_=p_ow[:])
            s_qbp = sb.tile([128, H], f32)
            nc.sync.dma_start(out=s_qbp, in_=p_qbp[:])
            s_gbn = sb.tile([128, 2], f32)
            nc.sync.dma_start(out=s_gbn, in_=p_gbn[:])
            s_kb = sb.tile([128, 2], f32)
            nc.sync.dma_start(out=s_kb, in_=p_kb[:])
            s_vbb = sb.tile([128, 256], f32)
            nc.sync.dma_start(out=s_vbb, in_=p_vbb[:])
            s_ob = sb.tile([128, 2], f32)
            nc.sync.dma_start(out=s_ob, in_=p_ob[:])
            s_iden = sb.tile([128, 128], bf16)
            nc.sync.dma_start(out=s_iden, in_=p_iden[:])
            s_ind = sb.tile([128, 128], bf16)
            nc.sync.dma_start(out=s_ind, in_=p_ind[:])

            s_ones = sb.tile([128, 1], bf16)
            nc.vector.memset(s_ones, 1.0)
            s_zcol = sb.tile([1, 128], bf16)
            nc.vector.memset(s_zcol, 0.0)
            s_zrow = sb.tile([1, 512], bf16)
            nc.vector.memset(s_zrow, 0.0)

            # bias, streamed in 4 big chunks ordered by consumption
            s_bT = sb.tile([128, NG, NKC, HPG * QS], bf16)
            for g in range(NG):
                for half in range(2):
                    c0 = half * (NKC // 2)
                    nc.sync.dma_start(
                        out=s_bT[:, g, c0:c0 + NKC // 2, :],
                        in_=p_bT[:, g, c0:c0 + NKC // 2, :],
                    )

            # ---- qg projection -> per-head padded qT (bf16) and sigT (f32) ----
            s_qT = sb.tile([128, H, QS], bf16)
            s_sigT = sb.tile([128, 2, QS], f32)
            for h in range(H):
                pt = pswork.tile([128, 512], f32, tag="work", name=f"q_ps_{h}")
                for kc in range(2):
                    nc.tensor.matmul(
                        pt[:, :QS], lhsT=s_wqp[:, h, kc, :], rhs=s_qiT[:, kc, :],
                        start=(kc == 0), stop=(kc == 1),
                    )
                nc.vector.tensor_scalar_add(s_qT[:, h, :], pt[:, :QS], s_qbp[:, h:h + 1])
            for g in range(NG):
                pt = pswork.tile([128, 512], f32, tag="work", name=f"g_ps_{g}")
                for kc in range(2):
                    nc.tensor.matmul(
                        pt[:, :QS], lhsT=s_wg[:, g, kc, :], rhs=s_qiT[:, kc, :],
                        start=(kc == 0), stop=(kc == 1),
                    )
                tt = tmp.tile([128, QS], f32, tag="sigtmp", name=f"sig_{g}")
                nc.scalar.activation(tt, pt[:, :QS], Exp, bias=s_gbn[:, g:g + 1], scale=-1.0)
                nc.vector.tensor_scalar_add(tt, tt, 1.0)
                nc.vector.reciprocal(s_sigT[:, g, :], tt)

            # ---- kT projection (bf16) ----
            s_kT = sb.tile([128, 2, KVL], bf16)
            for t in range(2):
                for ns in range(4):
                    pt = pswork.tile([128, 512], f32, tag="work", name=f"kt_ps_{t}_{ns}")
                    for kc in range(2):
                        nc.tensor.matmul(
                            pt, lhsT=s_wk[:, t, kc, :],
                            rhs=s_kviT[:, kc, ns * 512:(ns + 1) * 512],
                            start=(kc == 0), stop=(kc == 1),
                        )
                    nc.vector.tensor_scalar_add(
                        s_kT[:, t, ns * 512:(ns + 1) * 512], pt, s_kb[:, t:t + 1])

            # ---- v projection (bf16) ----
            s_v = sb.tile([128, NKC, H * C], bf16)
            for c in range(NKC):
                pt = pswork.tile([128, 512], f32, tag="work", name=f"v_ps_{c}")
                for kc in range(2):
                    nc.tensor.matmul(
                        pt[:, :256], lhsT=s_kviT[:, kc, c * 128:(c + 1) * 128],
                        rhs=s_wv[:, kc, :],
                        start=(kc == 0), stop=(kc == 1),
                    )
                nc.vector.tensor_tensor(s_v[:, c, :], pt[:, :256], s_vbb, ADD)

            # ---- attention, one head-group at a time ----
            s_agT = sb.tile([128, 2, QS], bf16)
            for g in range(NG):
                acc = ps.tile([128, 512], f32, tag="accum", name=f"acc_{g}")
                # zero the whole accumulator bank and set has_written bits
                nc.tensor.matmul(acc, lhsT=s_zcol, rhs=s_zrow, start=True, stop=False,
                                 skip_group_check=True)
                for c in range(NKC):
                    lt = ps.tile([128, HPG, QS], f32, tag="lt", name=f"lt_{g}_{c}")
                    # bias inject: one matmul per PSUM bank (2 heads each),
                    # start=True clears the bank's has_written bits once.
                    for b2 in range(2):
                        nc.tensor.matmul(
                            lt[:, 2 * b2:2 * b2 + 2, :], lhsT=s_iden,
                            rhs=s_bT[:, g, c, 512 * b2:512 * (b2 + 1)],
                            start=True, stop=False, tile_position=(0, 0),
                            skip_group_check=True,
                        )
                    # q.k^T: full-K matmul against the packed k tile; the
                    # per-head zero-padded qT keeps heads separated.
                    for hp in range(HPG):
                        h = HPG * g + hp
                        nc.tensor.matmul(
                            lt[:, hp, :],
                            lhsT=s_kT[:, g, c * 128:(c + 1) * 128],
                            rhs=s_qT[:, h, :],
                            start=False, stop=True,
                            skip_group_check=True,
                        )
                    et = etp.tile([128, HPG, QS], bf16, tag="et", name=f"et_{g}_{c}")
                    for b2 in range(2):  # ACT must not cross PSUM banks
                        nc.scalar.activation(et[:, 2 * b2:2 * b2 + 2, :],
                                             lt[:, 2 * b2:2 * b2 + 2, :], Exp)
                    # attn @ v (col-packed) + rowsum via ones column
                    for hp in range(HPG):
                        h = HPG * g + hp
                        nc.tensor.matmul(
                            acc[32 * hp:32 * hp + 32, 0:QS],
                            lhsT=s_v[:, c, 32 * h:32 * h + 32], rhs=et[:, hp, :],
                            start=False, stop=(c == NKC - 1),
                            tile_position=(0, 32 * hp), skip_group_check=True,
                        )
                        nc.tensor.matmul(
                            acc[32 * hp:32 * hp + 1, QS:2 * QS],
                            lhsT=s_ones, rhs=et[:, hp, :],
                            start=False, stop=(c == NKC - 1),
                            tile_position=(0, 32 * hp), skip_group_check=True,
                        )
                # softmax denominator: gather -> broadcast -> reciprocal
                rsg = tmp.tile([128, QS], bf16, tag="rsg", name=f"rsg_{g}")
                nc.vector.memset(rsg, 0.0)
                for hp in range(HPG):
                    nc.vector.tensor_copy(
                        out=rsg[32 * hp:32 * hp + 1, :],
                        in_=acc[32 * hp:32 * hp + 1, QS:2 * QS])
                rsb = pswork.tile([128, 512], f32, tag="work", name=f"rsb_{g}")
                nc.tensor.matmul(rsb[:, :QS], lhsT=s_ind, rhs=rsg, start=True, stop=True)
                recipB = tmp.tile([128, QS], f32, tag="recip", name=f"recip_{g}")
                nc.vector.reciprocal(recipB, rsb[:, :QS])
                gt1 = tmp.tile([128, QS], f32, tag="gt1", name=f"gt1_{g}")
                nc.vector.tensor_tensor(gt1, acc[:, 0:QS], s_sigT[:, g, :], MUL)
                nc.vector.tensor_tensor(s_agT[:, g, :], gt1, recipB, MUL)

            # ---- output projection ----
            s_outT = sb.tile([128, 2, QS], f32)
            for t in range(2):
                pt = pswork.tile([128, 512], f32, tag="work", name=f"o_ps_{t}")
                for g in range(2):
                    nc.tensor.matmul(
                        pt[:, :QS], lhsT=s_ow[:, g, t, :], rhs=s_agT[:, g, :],
                        start=(g == 0), stop=(g == 1),
                    )
                nc.scalar.add(s_outT[:, t, :], pt[:, :QS], s_ob[:, t:t + 1])
                nc.sync.dma_start(out=p_out[t], in_=s_outT[:, t, :])

    nc.finalize()
    return nc


_NC = None


def _get_nc():
    global _NC
    if _NC is None:
        _NC = build_kernel()
    return _NC


def kernel(**inputs) -> np.ndarray:
    nc = _get_nc()
    in_maps = make_in_maps(inputs)
    res = run_bass_kernel_spmd(nc, in_maps, core_ids=list(range(NCORES)))
    return gather_output(res.results)


def kernel_traced(**inputs):
    """Like kernel() but with NTFF profiling; returns (output, exec_time_ns)."""
    nc = _get_nc()
    in_maps = make_in_maps(inputs)
    res = run_bass_kernel_spmd(nc, in_maps, core_ids=list(range(NCORES)), trace=True)
    return gather_output(res.results), res.exec_time_ns, res
